# revision 4
# baseline (speedup 1.0000x reference)
"""Kalman filter (T=100000, dz=dy=16) on 8 Trainium2 NeuronCores.

Strategy:
  The covariance recursion is observation-independent and converges to a
  steady state in ~15 steps, after which the filter is a linear
  time-invariant system: fm_t = G fm_{t-1} + K obs_t + d0 with
  spectral_radius(G) ~= 0.42, so G^j vanishes (< 1e-12) for j >= ~33.
  The filtered means for t >= 128 are therefore a 40-tap causal
  convolution over observations, and the log-likelihood innovations are
  a similar convolution; both are evaluated on-device as dense matmuls
  in a (t mod 8, channel)-on-partitions layout (block-Toeplitz 128x128
  weights, 5 accumulating matmuls per output tile). Covariances for
  t >= 128 are a constant tile broadcast-written to HBM. The host
  computes the first 128 steps exactly (tiny) plus the steady-state
  matrices, and assembles the final outputs.
"""

import numpy as np

import concourse.bacc as bacc
import concourse.bass as bass
import concourse.mybir as mybir
from concourse.bass import MemorySpace
from concourse.bass_utils import run_bass_kernel_spmd
from concourse.tile import TileContext

F32 = mybir.dt.float32

T, DY, DZ = 100000, 16, 16
T0 = 128            # timesteps computed on host
NCORES = 8
CCOLS = 1568        # per-core output columns (1 column = 8 timesteps)
INCOLS = CCOLS + 8  # halo of 8 columns (64 timesteps) on the left
NF = 392            # matmul tile width (4 tiles per core)
NTILES = CCOLS // NF
NDELTA = 5          # column shifts 0..4 -> taps up to j = 39
NTAP = 40
COV_ROWS = (T - T0) // NCORES   # 12484 steady covariance rows per core
LOG2PI = float(np.log(2.0 * np.pi))

_NC_CACHE = {}


def _build_nc():
    nc = bacc.Bacc(
        "TRN2",
        target_bir_lowering=False,
        debug=False,
        num_devices=NCORES,
    )

    obs_in = nc.dram_tensor("obs_in", [INCOLS, 128], F32, kind="ExternalInput")
    wfm_in = nc.dram_tensor("wfm_in", [NDELTA * 128, 128], F32, kind="ExternalInput")
    wll_in = nc.dram_tensor("wll_in", [NDELTA * 128, 128], F32, kind="ExternalInput")
    beta_in = nc.dram_tensor("beta_in", [128, 1], F32, kind="ExternalInput")
    ngamma_in = nc.dram_tensor("ngamma_in", [128, 1], F32, kind="ExternalInput")
    ones_in = nc.dram_tensor("ones_in", [128, 1], F32, kind="ExternalInput")
    ident_in = nc.dram_tensor("ident_in", [128, 128], F32, kind="ExternalInput")
    cov_in = nc.dram_tensor("cov_in", [128, 1024], F32, kind="ExternalInput")

    means_out = nc.dram_tensor("means_out", [CCOLS, 128], F32, kind="ExternalOutput")
    covs_out = nc.dram_tensor("covs_out", [COV_ROWS, 256], F32, kind="ExternalOutput")
    ll_out = nc.dram_tensor("ll_out", [1, CCOLS], F32, kind="ExternalOutput")

    with TileContext(nc) as tc:
        with (
            tc.tile_pool(name="consts", bufs=1) as consts,
            tc.tile_pool(name="masters", bufs=1) as masters,
            tc.tile_pool(name="work", bufs=3) as work,
            tc.tile_pool(name="ptp", bufs=2, space=MemorySpace.PSUM) as ptp,
            tc.tile_pool(name="pfm", bufs=2, space=MemorySpace.PSUM) as pfmp,
            tc.tile_pool(name="pll", bufs=2, space=MemorySpace.PSUM) as pllp,
            tc.tile_pool(name="psum_s", bufs=2, space=MemorySpace.PSUM) as psp,
        ):
            # ---- constants ----
            ident = consts.tile([128, 128], F32)
            nc.sync.dma_start(ident[:], ident_in[:])
            wfm = consts.tile([128, NDELTA, 128], F32)
            nc.sync.dma_start(wfm[:], wfm_in.rearrange("(d k) m -> k d m", k=128))
            wll = consts.tile([128, NDELTA, 128], F32)
            nc.sync.dma_start(wll[:], wll_in.rearrange("(d k) m -> k d m", k=128))
            beta = consts.tile([128, 1], F32)
            nc.sync.dma_start(beta[:], beta_in[:])
            ngamma = consts.tile([128, 1], F32)
            nc.sync.dma_start(ngamma[:], ngamma_in[:])
            ones = consts.tile([128, 1], F32)
            nc.sync.dma_start(ones[:], ones_in[:])
            covsb = consts.tile([128, 1024], F32)
            nc.sync.dma_start(covsb[:], cov_in[:])

            # ---- constant covariance broadcast (bulk of the HBM traffic) ----
            nfull = COV_ROWS // 512          # 24 full 512-row chunks
            for i in range(nfull):
                nc.sync.dma_start(
                    covs_out[i * 512 : (i + 1) * 512, :].rearrange(
                        "(p k) c -> p (k c)", p=128
                    ),
                    covsb[:],
                )
            rem = COV_ROWS - nfull * 512     # 196 rows = 49 partitions x 4
            nc.sync.dma_start(
                covs_out[nfull * 512 :, :].rearrange("(p k) c -> p (k c)", p=rem // 4),
                covsb[: rem // 4, :],
            )

            # ---- load observations and transpose to (128, INCOLS) layout ----
            # obs_in viewed as Z[col, 128] with Z[col, 16r+e] = obs[8 col + r, e];
            # we need OT8 = Z^T in SBUF.
            ngrp = INCOLS // 128             # 12 full 128-column groups
            ztail_cols = INCOLS - ngrp * 128  # 40
            znat = masters.tile([128, ngrp, 128], F32)
            nc.sync.dma_start(
                znat[:], obs_in[: ngrp * 128, :].rearrange("(g p) c -> p g c", p=128)
            )
            ztail = masters.tile([ztail_cols, 128], F32)
            nc.sync.dma_start(ztail[:], obs_in[ngrp * 128 :, :])

            ot8 = masters.tile([128, INCOLS], F32)
            for g in range(ngrp):
                pt = ptp.tile([128, 128], F32, tag="pt")
                nc.tensor.transpose(pt[:], znat[:, g, :], ident[:])
                nc.vector.tensor_copy(ot8[:, g * 128 : (g + 1) * 128], pt[:])
            pt = ptp.tile([128, ztail_cols], F32, tag="pt")
            nc.tensor.transpose(pt[:], ztail[:], ident[:ztail_cols, :ztail_cols])
            nc.vector.tensor_copy(ot8[:, ngrp * 128 :], pt[:])

            # ---- convolution matmuls ----
            fm8 = masters.tile([128, CCOLS], F32)
            llsb = masters.tile([1, CCOLS], F32)
            for jt in range(NTILES):
                base = 8 + jt * NF
                pfm = pfmp.tile([128, NF], F32, tag="pfm")
                for d in range(NDELTA):
                    nc.tensor.matmul(
                        pfm[:],
                        wfm[:, d, :],
                        ot8[:, base - d : base - d + NF],
                        start=(d == 0),
                        stop=(d == NDELTA - 1),
                    )
                nc.scalar.activation(
                    fm8[:, jt * NF : (jt + 1) * NF],
                    pfm[:],
                    mybir.ActivationFunctionType.Identity,
                    bias=beta[:],
                )
                pll = pllp.tile([128, NF], F32, tag="pll")
                for d in range(NDELTA):
                    nc.tensor.matmul(
                        pll[:],
                        wll[:, d, :],
                        ot8[:, base - d : base - d + NF],
                        start=(d == 0),
                        stop=(d == NDELTA - 1),
                    )
                dsq = work.tile([128, NF], F32, tag="dsq")
                nc.scalar.activation(
                    dsq[:],
                    pll[:],
                    mybir.ActivationFunctionType.Square,
                    bias=ngamma[:],
                )
                ps = psp.tile([1, NF], F32, tag="ps")
                nc.tensor.matmul(ps[:], ones[:], dsq[:], start=True, stop=True)
                nc.vector.tensor_copy(llsb[:, jt * NF : (jt + 1) * NF], ps[:])
            nc.sync.dma_start(ll_out[:], llsb[:])

            # ---- transpose means back to natural layout and store ----
            ogrp = CCOLS // 128              # 12 full groups
            otail_cols = CCOLS - ogrp * 128  # 32
            onat = masters.tile([128, ogrp, 128], F32)
            for g in range(ogrp):
                pt = ptp.tile([128, 128], F32, tag="pt")
                nc.tensor.transpose(pt[:], fm8[:, g * 128 : (g + 1) * 128], ident[:])
                nc.vector.tensor_copy(onat[:, g, :], pt[:])
            otail = masters.tile([otail_cols, 128], F32)
            pt = ptp.tile([otail_cols, 128], F32, tag="pt")
            nc.tensor.transpose(pt[:], fm8[:, ogrp * 128 :], ident[:])
            nc.vector.tensor_copy(otail[:], pt[:])

            nc.sync.dma_start(
                means_out[: ogrp * 128, :].rearrange("(g p) c -> p g c", p=128),
                onat[:],
            )
            nc.sync.dma_start(means_out[ogrp * 128 :, :], otail[:])

    nc.compile()
    return nc


def _host_setup(observations, transition_matrix, transition_offset, transition_cov,
                emission_matrix, emission_offset, emission_cov, prior_mean, prior_cov):
    """Exact prefix + steady-state matrices + device weights, in float64."""
    obs = observations.astype(np.float64)
    F = transition_matrix.astype(np.float64)
    b = transition_offset.astype(np.float64)
    q = transition_cov.astype(np.float64)
    H = emission_matrix.astype(np.float64)
    c = emission_offset.astype(np.float64)
    r = emission_cov.astype(np.float64)
    m0 = prior_mean.astype(np.float64)
    p0 = prior_cov.astype(np.float64)

    Q = np.diag(q ** 2)
    R = np.diag(r ** 2)

    pre_means = np.zeros((T0, DZ))
    pre_covs = np.zeros((T0, DZ, DZ))
    pre_ll = 0.0
    pm, pP = m0, np.diag(p0 ** 2)
    for t in range(T0):
        S = H @ pP @ H.T + R
        K = pP @ np.linalg.solve(S, H).T
        innov = obs[t] - H @ pm - c
        fm = pm + K @ innov
        fP = pP - K @ H @ pP
        L = np.linalg.cholesky(S)
        d = np.linalg.solve(L, innov)
        pre_ll += -0.5 * DY * LOG2PI - np.sum(np.log(np.diag(L))) - 0.5 * d @ d
        pre_means[t] = fm
        pre_covs[t] = fP
        pm = F @ fm + b
        pP = F @ fP @ F.T + Q

    # steady state
    for _ in range(200):
        S_ss = H @ pP @ H.T + R
        K_ss = pP @ np.linalg.solve(S_ss, H).T
        fP_ss = pP - K_ss @ H @ pP
        pP = F @ fP_ss @ F.T + Q
    L_ss = np.linalg.cholesky(S_ss)
    Linv = np.linalg.inv(L_ss)
    G = (np.eye(DZ) - K_ss @ H) @ F
    d0 = (np.eye(DZ) - K_ss @ H) @ b - K_ss @ c

    C = np.zeros((NTAP, DZ, DZ))
    Gj = np.eye(DZ)
    for j in range(NTAP):
        C[j] = Gj @ K_ss
        Gj = Gj @ G
    beta = np.linalg.solve(np.eye(DZ) - G, d0)

    A1 = Linv @ H @ F
    E = np.zeros((NTAP, DY, DZ))
    E[0] = Linv
    for j in range(1, NTAP):
        E[j] = -A1 @ C[j - 1]
    gamma = Linv @ (H @ b + c) + A1 @ beta
    CST = -0.5 * DY * LOG2PI - np.sum(np.log(np.diag(L_ss)))

    Wfm = np.zeros((NDELTA, 128, 128), dtype=np.float32)
    Wll = np.zeros((NDELTA, 128, 128), dtype=np.float32)
    for dl in range(NDELTA):
        for rr in range(8):
            for rp in range(8):
                j = rr - rp + 8 * dl
                if 0 <= j < NTAP:
                    Wfm[dl, rp * 16 : (rp + 1) * 16, rr * 16 : (rr + 1) * 16] = \
                        C[j].T.astype(np.float32)
                    Wll[dl, rp * 16 : (rp + 1) * 16, rr * 16 : (rr + 1) * 16] = \
                        E[j].T.astype(np.float32)

    return dict(
        pre_means=pre_means.astype(np.float32),
        pre_covs=pre_covs.astype(np.float32),
        pre_ll=pre_ll,
        fP_ss=fP_ss.astype(np.float32),
        Wfm=Wfm, Wll=Wll,
        beta128=np.tile(beta, 8).astype(np.float32).reshape(128, 1),
        ngamma128=np.tile(-gamma, 8).astype(np.float32).reshape(128, 1),
        CST=CST,
    )


def kernel(observations, transition_matrix, transition_offset, transition_cov,
           emission_matrix, emission_offset, emission_cov, prior_mean, prior_cov,
           _collect_perf=None):
    hs = _host_setup(observations, transition_matrix, transition_offset,
                     transition_cov, emission_matrix, emission_offset,
                     emission_cov, prior_mean, prior_cov)

    if "nc" not in _NC_CACHE:
        _NC_CACHE["nc"] = _build_nc()
    nc = _NC_CACHE["nc"]

    # per-core inputs
    obs_pad = np.zeros((8 * (16 + CCOLS * 8), 16), dtype=np.float32)
    obs_pad[:T] = observations.astype(np.float32)
    ident = np.eye(128, dtype=np.float32)
    ones = np.ones((128, 1), dtype=np.float32)
    cov_block = np.tile(hs["fP_ss"].reshape(1, 256), (128, 4)).astype(np.float32)
    wfm_flat = hs["Wfm"].reshape(NDELTA * 128, 128)
    wll_flat = hs["Wll"].reshape(NDELTA * 128, 128)

    in_maps = []
    for core in range(NCORES):
        colstart = 16 + CCOLS * core
        rows0 = 8 * (colstart - 8)
        chunk = obs_pad[rows0 : rows0 + 8 * INCOLS].reshape(INCOLS, 128)
        in_maps.append({
            "obs_in": np.ascontiguousarray(chunk),
            "wfm_in": wfm_flat, "wll_in": wll_flat,
            "beta_in": hs["beta128"], "ngamma_in": hs["ngamma128"],
            "ones_in": ones, "ident_in": ident, "cov_in": cov_block,
        })

    res = run_bass_kernel_spmd(
        nc, in_maps, core_ids=list(range(NCORES)),
        **(_collect_perf or {}),
    )
    if _collect_perf is not None:
        _NC_CACHE["last_results"] = res
    outs = res.results

    # ---- assemble ----
    means = np.empty((T, DZ), dtype=np.float32)
    covs = np.empty((T, DZ, DZ), dtype=np.float32)
    means[:T0] = hs["pre_means"]
    covs[:T0] = hs["pre_covs"]
    ll_tail = 0.0
    for core in range(NCORES):
        colstart = 16 + CCOLS * core
        t_lo = 8 * colstart
        mnat = outs[core]["means_out"].reshape(CCOLS * 8, DZ)
        n_valid = min(CCOLS * 8, T - t_lo)
        means[t_lo : t_lo + n_valid] = mnat[:n_valid]
        covs[T0 + COV_ROWS * core : T0 + COV_ROWS * (core + 1)] = \
            outs[core]["covs_out"].reshape(COV_ROWS, DZ, DZ)
        nvc = min(CCOLS, (T // 8) - colstart)
        part = outs[core]["ll_out"][0, :nvc].astype(np.float64)
        ll_tail += 8 * nvc * hs["CST"] - 0.5 * part.sum()

    ll = np.float32(hs["pre_ll"] + ll_tail)
    return means, covs, ll
